# revision 2
# baseline (speedup 1.0000x reference)
"""TRN2 Bass kernel for CompressedCausalAttention (batch-parallel, 8 cores).

Per-core dataflow (one batch element per NeuronCore):
  x^T, pe^T arrive pre-transposed from the host as [128, NK*S] bf16 chunk
  layouts; xpeT = x^T + pe^T via per-chunk DVE adds (no PE transposes).
  qkT [2C', S] = W_qk^T @ xpeT in bf16 (q bias folded into the PSUM->SBUF
  copy; k bias dropped entirely -- per-row-constant score shifts cancel in
  the softmax over t).
  V [S, C] bf16 with a ones column per head so the PV matmul accumulates
  softmax denominators in PSUM row 64 for free.
  Each pass p handles heads (2p, 2p+1) in two half-passes -- score columns
  [0,512) over t-blocks 0..3, then [512,1024) over t-blocks 0..7 -- so only
  two PV PSUM banks are live at once and the next pass's QK projection can
  run concurrently from its own PSUM pool (PSUM: 2 proj + 2x2 score + 2 PV
  banks). scores/exp/mask/PV all bf16: one exp per chunk covers both heads
  (strided 3D AP), gpsimd zeroes masked diagonal probs.
  Denominator reciprocals are taken directly off the PSUM row per
  (head, half); the rdenom broadcast over head rows happens via two rank-1
  0/1-row matmuls, deferred into the next pass's program order so the PSUM
  WAR never stalls. PSUM->SBUF copies are split across DVE (q, evict) and
  the scalar engine (k, V) to balance engine load; DMA issue is spread over
  the SP/Activation/Pool queues. Out projection in bf16; b_out (+ V-bias
  folded through W_out on the host) is added by the output copy against a
  host-broadcast [128, C] constant.
"""
import os
import numpy as np

import concourse.bass as bass
import concourse.bacc as bacc
import concourse.mybir as mybir
import concourse.tile as tile
from concourse.bass_utils import run_bass_kernel_spmd

S, B, C, H = 1024, 8, 768, 12
CC = C // H            # 64
NS = S // 128          # 8 s/t blocks
NK = C // 128          # 6 contraction chunks of 128
NKD = C // 256         # 3 DoubleRow chunks of 256
NM = 2 * C // 128      # 12 q+k M-tiles
F32 = mybir.dt.float32
BF16 = mybir.dt.bfloat16
FP8 = mybir.dt.float8e4
F32R = mybir.dt.float32r
REPEAT = int(os.environ.get("BASSK_REPEAT", "1"))
AF = mybir.ActivationFunctionType
ALU = mybir.AluOpType
DR = mybir.MatmulPerfMode.DoubleRow
WS = 64.0              # fp8 weight pre-scale
_CACHE = {}

# far-column chunks for the DR projections (N <= 256: rhs free = 2N <= 512)
FAR_CHUNKS = [(128, 384), (384, 640), (640, 896), (896, 1024)]


def _build(repeat=None):
    if repeat is None:
        repeat = REPEAT
    nc = bacc.Bacc("TRN2", target_bir_lowering=False, debug=False)

    Xb = nc.dram_tensor("xb", [128, NK * S], BF16, kind="ExternalInput")
    PEb = nc.dram_tensor("peb", [128, NK * S], BF16, kind="ExternalInput")
    Wqkb = nc.dram_tensor("wqkb", [128, NK * 2 * C], BF16, kind="ExternalInput")
    Wvb = nc.dram_tensor("wvb", [128, NK * C], BF16, kind="ExternalInput")
    Wo = nc.dram_tensor("wo", [128, NK * C], BF16, kind="ExternalInput")
    Bq = nc.dram_tensor("bq", [128, NK], F32, kind="ExternalInput")
    Beffb = nc.dram_tensor("beffb", [128, C], F32, kind="ExternalInput")
    Mask01 = nc.dram_tensor("mask01", [128, 128], BF16, kind="ExternalInput")
    E2 = nc.dram_tensor("e2", [2, 128], F32R, kind="ExternalInput")  # eA/eB rows
    OnesV = nc.dram_tensor("onesv", [128, H], BF16, kind="ExternalInput")
    Y = nc.dram_tensor("y", [S, C], F32, kind="ExternalOutput")

    from contextlib import ExitStack
    with ExitStack() as _es:
        tc = _es.enter_context(tile.TileContext(nc))
        _p = lambda **kw: _es.enter_context(tc.tile_pool(**kw))
        cst = _p(name="cst", bufs=1)
        xin_p = _p(name="xin", bufs=1)
        qkT_p = _p(name="qkT", bufs=5)
        vx_p = _p(name="vx", bufs=NS)
        pT_p = _p(name="pT", bufs=4)
        pvT_p = _p(name="pvT", bufs=NK)
        ysb_p = _p(name="ysb", bufs=2)
        dstage_p = _p(name="dstage", bufs=2)
        ps1 = _p(name="ps1", bufs=2, space="PSUM")    # 2 x 1-bank slots
        scps = _p(name="scps", bufs=2, space="PSUM")  # 2 x 2-bank slots
        pvps = _p(name="pvps", bufs=2, space="PSUM")  # 2 x 1-bank slots

        def go():
            mask_sb = cst.tile([128, 128], BF16, tag="mask", name="mask_sb")
            bq_sb = cst.tile([128, NK], F32, tag="bq", name="bq_sb")
            beff_sb = cst.tile([128, C], F32, tag="beff", name="beff_sb")
            eA_sb = cst.tile([1, 128], F32R, tag="eA", name="eA_sb")
            eB_sb = cst.tile([1, 128], F32R, tag="eB", name="eB_sb")

            # ---- inputs: full bf16 transposed x/pe; add per k-chunk on DVE
            xb = xin_p.tile([128, NK * S], BF16, tag="xb", name="xb")
            peb = xin_p.tile([128, NK * S], BF16, tag="peb", name="peb")
            for kc in range(NK):
                sl = slice(kc * S, (kc + 1) * S)
                nc.scalar.dma_start(xb[:, sl], Xb.ap()[:, sl])
                nc.scalar.dma_start(peb[:, sl], PEb.ap()[:, sl])
                nc.vector.tensor_tensor(xb[:, sl], xb[:, sl], peb[:, sl],
                                        ALU.add)
            xpeb = [xb[:, kc * S:(kc + 1) * S] for kc in range(NK)]

            # ---- weights
            wqkb_sb = cst.tile([128, NK * 2 * C], BF16, tag="wqkb", name="wqkb_sb")
            nc.sync.dma_start(wqkb_sb[:], Wqkb.ap())
            wvb_sb = cst.tile([128, NK * C], BF16, tag="wvb", name="wvb_sb")
            nc.gpsimd.dma_start(wvb_sb[:], Wvb.ap())
            wo_sb = cst.tile([128, NK * C], BF16, tag="wo", name="wo_sb")
            nc.sync.dma_start(wo_sb[:], Wo.ap())
            nc.gpsimd.dma_start(mask_sb[:], Mask01.ap())
            nc.gpsimd.dma_start(bq_sb[:], Bq.ap())
            nc.sync.dma_start(beff_sb[:], Beffb.ap())
            nc.sync.dma_start(eA_sb[:], E2.ap()[0:1, :])
            nc.sync.dma_start(eB_sb[:], E2.ap()[1:2, :])
            wqkbv = [wqkb_sb[:, kc * 2 * C:(kc + 1) * 2 * C] for kc in range(NK)]
            wvbv = [wvb_sb[:, kc * C:(kc + 1) * C] for kc in range(NK)]
            wov = [wo_sb[:, kc * C:(kc + 1) * C] for kc in range(NK)]

            # ---- Phase A2: V (near: block 0 bf16; far: blocks 1..7 fp8 DR)
            vx = []
            for si in range(NS):
                v = vx_p.tile([128, H * (CC + 1)], BF16, tag="vx", name=f"vx{si}")
                v3 = v[:].rearrange("p (h c) -> p h c", h=H)

                def vcopy(ps_, c0, c1, scale):
                    h0, h1 = c0 // CC, c1 // CC
                    nc.scalar.activation(
                        v3[:, h0:h1, 0:CC],
                        ps_[:].rearrange("p (h c) -> p h c", h=h1 - h0),
                        AF.Copy, scale=scale)
                for (c0, c1) in ((0, 512), (512, C)):
                    vp = ps1.tile([128, c1 - c0], F32, tag="ps1",
                                  name=f"vp{si}_{c0}")
                    for k in range(NK):
                        nc.tensor.matmul(
                            vp[:], xpeb[k][:, si * 128:(si + 1) * 128],
                            wvbv[k][:, c0:c1],
                            start=(k == 0), stop=(k == NK - 1))
                    vcopy(vp, c0, c1, 1.0)
                nc.gpsimd.dma_start(v3[:, :, CC:CC + 1], OnesV.ap()[:, :, None])
                vx.append(v)

            # ---- Phases B+C: per pass p: qkT tiles (p, 6+p), heads 2p, 2p+1
            pvT = [pvT_p.tile([128, S], BF16, tag="pvT", name=f"pvT{j}")
                   for j in range(NK)]
            def run_norm(job):
                # bc + multiply for a finished pass; placed after the next
                # pass's projection allocs so the ps1 WAR hits a fast slot
                p_, rden1_ = job
                for n2 in (0, 1):
                    bc = ps1.tile([128, 512], F32, tag="ps1",
                                  name=f"bc{p_}_{n2}")
                    nc.tensor.matmul(
                        bc[:], eA_sb[:],
                        rden1_[:, n2 * 512:(n2 + 1) * 512],
                        start=True, stop=False)
                    nc.tensor.matmul(
                        bc[:], eB_sb[:],
                        rden1_[:, S + n2 * 512:S + (n2 + 1) * 512],
                        start=False, stop=True)
                    nc.vector.tensor_tensor(
                        pvT[p_][:, n2 * 512:(n2 + 1) * 512],
                        pvT[p_][:, n2 * 512:(n2 + 1) * 512], bc[:], ALU.mult)

            norm_job = None
            for p in range(6):
                qk = {}
                for mm in (p, 6 + p):
                    t = qkT_p.tile([128, S], BF16, tag="qkT", name=f"qkT{mm}")
                    is_q = mm < NK
                    for (a0, a1) in ((0, 512), (512, S)):
                        psf = ps1.tile([128, 512], F32, tag="ps1",
                                       name=f"qf{mm}_{a0}")
                        for k in range(NK):
                            nc.tensor.matmul(
                                psf[:], wqkbv[k][:, mm * 128:(mm + 1) * 128],
                                xpeb[k][:, a0:a1],
                                start=(k == 0), stop=(k == NK - 1))
                        if is_q:
                            nc.vector.tensor_scalar(
                                t[:, a0:a1], psf[:], bq_sb[:, mm:mm + 1],
                                None, ALU.add)
                        else:
                            nc.scalar.activation(t[:, a0:a1], psf[:], AF.Copy)
                    qk[mm] = t
                if norm_job is not None:
                    run_norm(norm_job)
                    norm_job = None
                qt, kt = qk[p], qk[6 + p]
                heads = (2 * p, 2 * p + 1)
                rden1 = dstage_p.tile([1, 2 * S], F32R, tag="rdst",
                                      name=f"rden1_{p}")

                def evict(pvt, h, half):
                    r0 = (h % 2) * CC
                    c0 = (h % 2) * S + half * 512
                    with nc.allow_low_precision(reason="bf16 softmax scale"):
                        nc.vector.reciprocal(rden1[:, c0:c0 + 512],
                                             pvt[CC:CC + 1, :])
                    nc.vector.tensor_copy(pvT[p][r0:r0 + CC,
                                                 half * 512:half * 512 + 512],
                                          pvt[0:CC, :])

                # two half-passes: cols [0,512) over Ti 0..3, then cols
                # [512,1024) over Ti 0..7 — only 2 PV banks live at a time
                for half, tis in ((0, range(4)), (1, range(NS))):
                    c0 = half * 512
                    pv = {h: pvps.tile([CC + 1, 512], F32, tag="pvps",
                                       name=f"pv{h}_{half}")
                          for h in heads}
                    for Ti in tis:
                        s0 = Ti * 128
                        a0 = max(s0, c0)
                        a1 = c0 + 512
                        w = a1 - a0
                        pt = pT_p.tile([128, 2 * w], BF16, tag="pT",
                                       name=f"pt{p}_{half}_{Ti}")
                        off = {heads[0]: 0, heads[1]: w}
                        sc = scps.tile([128, 1024], F32, tag="scps",
                                       name=f"sc{p}_{half}_{Ti}")
                        for hi, h in enumerate(heads):
                            r0 = (h % 2) * CC
                            nc.tensor.matmul(
                                sc[:, hi * 512:hi * 512 + w],
                                kt[r0:r0 + CC, s0:s0 + 128],
                                qt[r0:r0 + CC, a0:a1],
                                start=True, stop=True)
                        ap_out = pt[:].rearrange("q (i c) -> q i c", i=2)
                        ap_in = sc[:].rearrange(
                            "q (i c) -> q i c", i=2)[:, :, 0:w]
                        nc.scalar.activation(
                            ap_out, ap_in, AF.Exp, scale=float(1.0 / np.sqrt(CC)))
                        if a0 == s0:  # diagonal blocks: zero masked probs
                            for h in heads:
                                o = off[h]
                                nc.gpsimd.tensor_tensor(
                                    pt[:, o:o + 128], pt[:, o:o + 128],
                                    mask_sb[:], ALU.mult)
                        for h in heads:
                            v3 = vx[Ti][:].rearrange("p (h c) -> p h c", h=H)
                            nc.tensor.matmul(
                                pv[h][:, a0 - c0:a1 - c0], v3[:, h, :],
                                pt[:, off[h]:off[h] + w],
                                start=(Ti == tis[0]), stop=(Ti == tis[-1]))
                    for h in heads:
                        evict(pv[h], h, half)

                # bc deferred to next pass
                norm_job = (p, rden1)

            run_norm(norm_job)

            # ---- Phase D: output projection
            for si in range(NS):
                ty = ysb_p.tile([128, C], F32, tag="ysb", name=f"ty{si}")
                for (c0, c1) in ((0, 512), (512, C)):
                    yp = ps1.tile([128, c1 - c0], F32, tag="ps1",
                                  name=f"yp{si}_{c0}")
                    for k in range(NK):
                        nc.tensor.matmul(
                            yp[:], pvT[k][:, si * 128:(si + 1) * 128],
                            wov[k][:, c0:c1],
                            start=(k == 0), stop=(k == NK - 1))
                    nc.vector.tensor_tensor(ty[:, c0:c1], yp[:],
                                            beff_sb[:, c0:c1], ALU.add)
                nc.sync.dma_start(Y.ap()[si * 128:(si + 1) * 128, :], ty[:])

        for _rep in range(repeat):
            go()

    nc.compile()
    return nc


def _prep(inputs):
    np8 = mybir.dt.np(FP8)
    npb = mybir.dt.np(BF16)
    x = np.asarray(inputs["x"], np.float32)
    pe = np.asarray(inputs["pe"], np.float32)
    W_qkv = np.asarray(inputs["W_qkv"], np.float32)
    b_qkv = np.asarray(inputs["b_qkv"], np.float32)
    W_out = np.asarray(inputs["W_out"], np.float32)
    b_out = np.asarray(inputs["b_out"], np.float32)

    wqk = np.ascontiguousarray(W_qkv[:, :2 * C])          # [C, 2C]
    wv = np.ascontiguousarray(W_qkv[:, 2 * C:])           # [C, C]
    wqkb = wqk.reshape(NK, 128, 2 * C).transpose(1, 0, 2) \
        .reshape(128, NK * 2 * C).astype(npb)
    wvb = wv.reshape(NK, 128, C).transpose(1, 0, 2) \
        .reshape(128, NK * C).astype(npb)
    wo = W_out.reshape(NK, 128, C).transpose(1, 0, 2) \
        .reshape(128, NK * C).astype(npb)
    bq = np.ascontiguousarray(b_qkv[:C].reshape(NK, 128).T).astype(np.float32)
    beff = (b_qkv[2 * C:] @ W_out + b_out).astype(np.float32)
    beffb = np.ascontiguousarray(np.broadcast_to(beff[None, :], (128, C)))
    t = np.arange(128)
    mask01 = (t[:, None] <= t[None, :]).astype(npb)
    e2 = np.zeros((2, 128), np.float32)
    e2[0, :CC] = 1.0
    e2[1, CC:] = 1.0
    common = dict(wqkb=wqkb, wvb=wvb, wo=wo, bq=bq,
                  beffb=beffb, mask01=mask01, e2=e2.astype(np.float32),
                  onesv=np.ones((128, H), npb))

    in_maps = []
    for b in range(B):
        m = dict(common)
        xT = x[:, b, :].T                                  # [C, S]
        peT = pe[:, b, :].T
        m["xb"] = np.ascontiguousarray(
            xT.reshape(NK, 128, S).transpose(1, 0, 2)
            .reshape(128, NK * S)).astype(npb)
        m["peb"] = np.ascontiguousarray(
            peT.reshape(NK, 128, S).transpose(1, 0, 2)
            .reshape(128, NK * S)).astype(npb)
        in_maps.append(m)
    return in_maps


def _run(inputs, trace=False):
    if "nc" not in _CACHE:
        _CACHE["nc"] = _build()
    nc = _CACHE["nc"]
    in_maps = _prep(inputs)
    res = run_bass_kernel_spmd(nc, in_maps, core_ids=list(range(B)), trace=trace)
    out = np.empty((S, B, C), np.float32)
    for b in range(B):
        out[:, b, :] = res.results[b]["y"]
    return out, res


def kernel(**inputs):
    out, _ = _run(inputs, trace=False)
    return out


# revision 4
# speedup vs baseline: 1.0402x; 1.0402x over previous
"""TRN2 Bass kernel for CompressedCausalAttention (batch-parallel, 8 cores).

Per-core dataflow (one batch element per NeuronCore):
  x^T, pe^T arrive pre-transposed from the host as [128, NK*S] bf16 chunk
  layouts; xpeT = x^T + pe^T via per-chunk DVE adds (no PE transposes).
  qkT [2C', S] = W_qk^T @ xpeT in bf16 (q bias folded into the PSUM->SBUF
  copy; k bias dropped entirely -- per-row-constant score shifts cancel in
  the softmax over t).
  V [S, C] bf16 with a ones column per head so the PV matmul accumulates
  softmax denominators in PSUM row 64 for free.
  Each pass p handles heads (2p, 2p+1) in two half-passes -- score columns
  [0,512) over t-blocks 0..3, then [512,1024) over t-blocks 0..7 -- so only
  two PV PSUM banks are live at once and the next pass's QK projection can
  run concurrently from its own PSUM pool (PSUM: 2 proj + 2x2 score + 2 PV
  banks). scores/exp/mask/PV all bf16: one exp per chunk covers both heads
  (strided 3D AP), gpsimd zeroes masked diagonal probs.
  Denominator reciprocals are taken directly off the PSUM row per
  (head, half); the rdenom broadcast over head rows happens via two rank-1
  0/1-row matmuls, deferred into the next pass's program order so the PSUM
  WAR never stalls. PSUM->SBUF copies are split across DVE (q, evict) and
  the scalar engine (k, V) to balance engine load; DMA issue is spread over
  the SP/Activation/Pool queues. Out projection in bf16; b_out (+ V-bias
  folded through W_out on the host) is added by the output copy against a
  host-broadcast [128, C] constant.
"""
import os
import numpy as np

import concourse.bass as bass
import concourse.bacc as bacc
import concourse.mybir as mybir
import concourse.tile as tile
from concourse.bass_utils import run_bass_kernel_spmd

S, B, C, H = 1024, 8, 768, 12
CC = C // H            # 64
NS = S // 128          # 8 s/t blocks
NK = C // 128          # 6 contraction chunks of 128
NKD = C // 256         # 3 DoubleRow chunks of 256
NM = 2 * C // 128      # 12 q+k M-tiles
F32 = mybir.dt.float32
BF16 = mybir.dt.bfloat16
FP8 = mybir.dt.float8e4
F32R = mybir.dt.float32r
REPEAT = int(os.environ.get("BASSK_REPEAT", "1"))
AF = mybir.ActivationFunctionType
ALU = mybir.AluOpType
DR = mybir.MatmulPerfMode.DoubleRow
WS = 64.0              # fp8 weight pre-scale
_CACHE = {}

# far-column chunks for the DR projections (N <= 256: rhs free = 2N <= 512)
FAR_CHUNKS = [(128, 384), (384, 640), (640, 896), (896, 1024)]


def _build(repeat=None):
    if repeat is None:
        repeat = REPEAT
    nc = bacc.Bacc("TRN2", target_bir_lowering=False, debug=False)

    Xb = nc.dram_tensor("xb", [128, NK * S], BF16, kind="ExternalInput")
    PEb = nc.dram_tensor("peb", [128, NK * S], BF16, kind="ExternalInput")
    Wqkb = nc.dram_tensor("wqkb", [128, NK * 2 * C], BF16, kind="ExternalInput")
    Wvb = nc.dram_tensor("wvb", [128, NK * C], BF16, kind="ExternalInput")
    Wo = nc.dram_tensor("wo", [128, NK * C], BF16, kind="ExternalInput")
    Bq = nc.dram_tensor("bq", [128, NK], F32, kind="ExternalInput")
    Beffb = nc.dram_tensor("beffb", [128, C], F32, kind="ExternalInput")
    Mask01 = nc.dram_tensor("mask01", [128, 128], BF16, kind="ExternalInput")
    E2 = nc.dram_tensor("e2", [2, 128], F32R, kind="ExternalInput")  # eA/eB rows
    OnesV = nc.dram_tensor("onesv", [128, H], BF16, kind="ExternalInput")
    Y = nc.dram_tensor("y", [S, C], BF16, kind="ExternalOutput")

    from contextlib import ExitStack
    with ExitStack() as _es:
        tc = _es.enter_context(tile.TileContext(nc))
        _p = lambda **kw: _es.enter_context(tc.tile_pool(**kw))
        cst = _p(name="cst", bufs=1)
        xin_p = _p(name="xin", bufs=1)
        qkT_p = _p(name="qkT", bufs=5)
        vx_p = _p(name="vx", bufs=NS)
        pT_p = _p(name="pT", bufs=4)
        pvT_p = _p(name="pvT", bufs=NK)
        ysb_p = _p(name="ysb", bufs=2)
        dstage_p = _p(name="dstage", bufs=2)
        ps1 = _p(name="ps1", bufs=2, space="PSUM")    # 2 x 1-bank slots
        scps = _p(name="scps", bufs=2, space="PSUM")  # 2 x 2-bank slots
        pvps = _p(name="pvps", bufs=2, space="PSUM")  # 2 x 1-bank slots

        def go():
            mask_sb = cst.tile([128, 128], BF16, tag="mask", name="mask_sb")
            bq_sb = cst.tile([128, NK], F32, tag="bq", name="bq_sb")
            beff_sb = cst.tile([128, C], F32, tag="beff", name="beff_sb")
            eA_sb = cst.tile([1, 128], F32R, tag="eA", name="eA_sb")
            eB_sb = cst.tile([1, 128], F32R, tag="eB", name="eB_sb")

            # ---- inputs: full bf16 transposed x/pe; add per k-chunk on DVE
            xb = xin_p.tile([128, NK * S], BF16, tag="xb", name="xb")
            peb = xin_p.tile([128, NK * S], BF16, tag="peb", name="peb")
            for kc in range(NK):
                sl = slice(kc * S, (kc + 1) * S)
                nc.scalar.dma_start(xb[:, sl], Xb.ap()[:, sl])
                nc.sync.dma_start(peb[:, sl], PEb.ap()[:, sl])
                nc.vector.tensor_tensor(xb[:, sl], xb[:, sl], peb[:, sl],
                                        ALU.add)
            xpeb = [xb[:, kc * S:(kc + 1) * S] for kc in range(NK)]

            # ---- weights
            wqkb_sb = cst.tile([128, NK * 2 * C], BF16, tag="wqkb", name="wqkb_sb")
            nc.sync.dma_start(wqkb_sb[:], Wqkb.ap())
            wvb_sb = cst.tile([128, NK * C], BF16, tag="wvb", name="wvb_sb")
            nc.gpsimd.dma_start(wvb_sb[:], Wvb.ap())
            wo_sb = cst.tile([128, NK * C], BF16, tag="wo", name="wo_sb")
            nc.sync.dma_start(wo_sb[:], Wo.ap())
            nc.gpsimd.dma_start(mask_sb[:], Mask01.ap())
            nc.gpsimd.dma_start(bq_sb[:], Bq.ap())
            nc.sync.dma_start(beff_sb[:], Beffb.ap())
            nc.sync.dma_start(eA_sb[:], E2.ap()[0:1, :])
            nc.sync.dma_start(eB_sb[:], E2.ap()[1:2, :])
            wqkbv = [wqkb_sb[:, kc * 2 * C:(kc + 1) * 2 * C] for kc in range(NK)]
            wvbv = [wvb_sb[:, kc * C:(kc + 1) * C] for kc in range(NK)]
            wov = [wo_sb[:, kc * C:(kc + 1) * C] for kc in range(NK)]

            # ---- Phase A2: V (near: block 0 bf16; far: blocks 1..7 fp8 DR)
            vx = []
            for si in range(NS):
                v = vx_p.tile([128, H * (CC + 1)], BF16, tag="vx", name=f"vx{si}")
                v3 = v[:].rearrange("p (h c) -> p h c", h=H)

                def vcopy(ps_, c0, c1, scale):
                    h0, h1 = c0 // CC, c1 // CC
                    nc.scalar.activation(
                        v3[:, h0:h1, 0:CC],
                        ps_[:].rearrange("p (h c) -> p h c", h=h1 - h0),
                        AF.Copy, scale=scale)
                for (c0, c1) in ((0, 512), (512, C)):
                    vp = ps1.tile([128, c1 - c0], F32, tag="ps1",
                                  name=f"vp{si}_{c0}")
                    for k in range(NK):
                        nc.tensor.matmul(
                            vp[:], xpeb[k][:, si * 128:(si + 1) * 128],
                            wvbv[k][:, c0:c1],
                            start=(k == 0), stop=(k == NK - 1))
                    vcopy(vp, c0, c1, 1.0)
                nc.gpsimd.dma_start(v3[:, :, CC:CC + 1], OnesV.ap()[:, :, None])
                vx.append(v)

            # ---- Phases B+C: per pass p: qkT tiles (p, 6+p), heads 2p, 2p+1
            pvT = [pvT_p.tile([128, S], BF16, tag="pvT", name=f"pvT{j}")
                   for j in range(NK)]
            def run_norm(job):
                # bc + multiply for a finished pass; placed after the next
                # pass's projection allocs so the ps1 WAR hits a fast slot
                p_, rden1_ = job
                for n2 in (0, 1):
                    bc = ps1.tile([128, 512], F32, tag="ps1",
                                  name=f"bc{p_}_{n2}")
                    nc.tensor.matmul(
                        bc[:], eA_sb[:],
                        rden1_[:, n2 * 512:(n2 + 1) * 512],
                        start=True, stop=False)
                    nc.tensor.matmul(
                        bc[:], eB_sb[:],
                        rden1_[:, S + n2 * 512:S + (n2 + 1) * 512],
                        start=False, stop=True)
                    nc.vector.tensor_tensor(
                        pvT[p_][:, n2 * 512:(n2 + 1) * 512],
                        pvT[p_][:, n2 * 512:(n2 + 1) * 512], bc[:], ALU.mult)

            norm_job = None
            for p in range(6):
                qk = {}
                for mm in (p, 6 + p):
                    t = qkT_p.tile([128, S], BF16, tag="qkT", name=f"qkT{mm}")
                    is_q = mm < NK
                    for (a0, a1) in ((0, 512), (512, S)):
                        psf = ps1.tile([128, 512], F32, tag="ps1",
                                       name=f"qf{mm}_{a0}")
                        for k in range(NK):
                            nc.tensor.matmul(
                                psf[:], wqkbv[k][:, mm * 128:(mm + 1) * 128],
                                xpeb[k][:, a0:a1],
                                start=(k == 0), stop=(k == NK - 1))
                        if is_q:
                            nc.vector.tensor_scalar(
                                t[:, a0:a1], psf[:], bq_sb[:, mm:mm + 1],
                                None, ALU.add)
                        else:
                            nc.scalar.activation(t[:, a0:a1], psf[:], AF.Copy)
                    qk[mm] = t
                if norm_job is not None:
                    run_norm(norm_job)
                    norm_job = None
                qt, kt = qk[p], qk[6 + p]
                heads = (2 * p, 2 * p + 1)
                rden1 = dstage_p.tile([1, 2 * S], F32R, tag="rdst",
                                      name=f"rden1_{p}")

                def evict(pvt, h, half):
                    r0 = (h % 2) * CC
                    c0 = (h % 2) * S + half * 512
                    with nc.allow_low_precision(reason="bf16 softmax scale"):
                        nc.vector.reciprocal(rden1[:, c0:c0 + 512],
                                             pvt[CC:CC + 1, :])
                    nc.vector.tensor_copy(pvT[p][r0:r0 + CC,
                                                 half * 512:half * 512 + 512],
                                          pvt[0:CC, :])

                # two half-passes: cols [0,512) over Ti 0..3, then cols
                # [512,1024) over Ti 0..7 — only 2 PV banks live at a time
                for half, tis in ((0, range(4)), (1, range(NS))):
                    c0 = half * 512
                    pv = {h: pvps.tile([CC + 1, 512], F32, tag="pvps",
                                       name=f"pv{h}_{half}")
                          for h in heads}
                    for Ti in tis:
                        s0 = Ti * 128
                        a0 = max(s0, c0)
                        a1 = c0 + 512
                        w = a1 - a0
                        pt = pT_p.tile([128, 2 * w], BF16, tag="pT",
                                       name=f"pt{p}_{half}_{Ti}")
                        off = {heads[0]: 0, heads[1]: w}
                        sc = scps.tile([128, 1024], F32, tag="scps",
                                       name=f"sc{p}_{half}_{Ti}")
                        for hi, h in enumerate(heads):
                            r0 = (h % 2) * CC
                            nc.tensor.matmul(
                                sc[:, hi * 512:hi * 512 + w],
                                kt[r0:r0 + CC, s0:s0 + 128],
                                qt[r0:r0 + CC, a0:a1],
                                start=True, stop=True)
                        ap_out = pt[:].rearrange("q (i c) -> q i c", i=2)
                        ap_in = sc[:].rearrange(
                            "q (i c) -> q i c", i=2)[:, :, 0:w]
                        nc.scalar.activation(
                            ap_out, ap_in, AF.Exp, scale=float(1.0 / np.sqrt(CC)))
                        if a0 == s0:  # diagonal blocks: zero masked probs
                            for h in heads:
                                o = off[h]
                                nc.gpsimd.tensor_tensor(
                                    pt[:, o:o + 128], pt[:, o:o + 128],
                                    mask_sb[:], ALU.mult)
                        for h in heads:
                            v3 = vx[Ti][:].rearrange("p (h c) -> p h c", h=H)
                            nc.tensor.matmul(
                                pv[h][:, a0 - c0:a1 - c0], v3[:, h, :],
                                pt[:, off[h]:off[h] + w],
                                start=(Ti == tis[0]), stop=(Ti == tis[-1]))
                    for h in heads:
                        evict(pv[h], h, half)

                # bc deferred to next pass
                norm_job = (p, rden1)

            # ---- Phase D: output projection (last pass's norm slots in
            # after the first psum group's k<5 matmuls to hide its stall)
            for si in range(NS):
                ty = ysb_p.tile([128, C], BF16, tag="ysb", name=f"ty{si}")
                for (c0, c1) in ((0, 512), (512, C)):
                    yp = ps1.tile([128, c1 - c0], F32, tag="ps1",
                                  name=f"yp{si}_{c0}")
                    for k in range(NK):
                        if norm_job is not None and k == NK - 1:
                            run_norm(norm_job)
                            norm_job = None
                        nc.tensor.matmul(
                            yp[:], pvT[k][:, si * 128:(si + 1) * 128],
                            wov[k][:, c0:c1],
                            start=(k == 0), stop=(k == NK - 1))
                    nc.vector.tensor_tensor(ty[:, c0:c1], yp[:],
                                            beff_sb[:, c0:c1], ALU.add)
                    nc.sync.dma_start(
                        Y.ap()[si * 128:(si + 1) * 128, c0:c1], ty[:, c0:c1])

        for _rep in range(repeat):
            go()

    nc.compile()
    return nc


def _prep(inputs):
    np8 = mybir.dt.np(FP8)
    npb = mybir.dt.np(BF16)
    x = np.asarray(inputs["x"], np.float32)
    pe = np.asarray(inputs["pe"], np.float32)
    W_qkv = np.asarray(inputs["W_qkv"], np.float32)
    b_qkv = np.asarray(inputs["b_qkv"], np.float32)
    W_out = np.asarray(inputs["W_out"], np.float32)
    b_out = np.asarray(inputs["b_out"], np.float32)

    wqk = np.ascontiguousarray(W_qkv[:, :2 * C])          # [C, 2C]
    wv = np.ascontiguousarray(W_qkv[:, 2 * C:])           # [C, C]
    wqkb = wqk.reshape(NK, 128, 2 * C).transpose(1, 0, 2) \
        .reshape(128, NK * 2 * C).astype(npb)
    wvb = wv.reshape(NK, 128, C).transpose(1, 0, 2) \
        .reshape(128, NK * C).astype(npb)
    wo = W_out.reshape(NK, 128, C).transpose(1, 0, 2) \
        .reshape(128, NK * C).astype(npb)
    bq = np.ascontiguousarray(b_qkv[:C].reshape(NK, 128).T).astype(np.float32)
    beff = (b_qkv[2 * C:] @ W_out + b_out).astype(np.float32)
    beffb = np.ascontiguousarray(np.broadcast_to(beff[None, :], (128, C)))
    t = np.arange(128)
    mask01 = (t[:, None] <= t[None, :]).astype(npb)
    e2 = np.zeros((2, 128), np.float32)
    e2[0, :CC] = 1.0
    e2[1, CC:] = 1.0
    common = dict(wqkb=wqkb, wvb=wvb, wo=wo, bq=bq,
                  beffb=beffb, mask01=mask01, e2=e2.astype(np.float32),
                  onesv=np.ones((128, H), npb))

    in_maps = []
    for b in range(B):
        m = dict(common)
        xT = x[:, b, :].T                                  # [C, S]
        peT = pe[:, b, :].T
        m["xb"] = np.ascontiguousarray(
            xT.reshape(NK, 128, S).transpose(1, 0, 2)
            .reshape(128, NK * S)).astype(npb)
        m["peb"] = np.ascontiguousarray(
            peT.reshape(NK, 128, S).transpose(1, 0, 2)
            .reshape(128, NK * S)).astype(npb)
        in_maps.append(m)
    return in_maps


def _run(inputs, trace=False):
    if "nc" not in _CACHE:
        _CACHE["nc"] = _build()
    nc = _CACHE["nc"]
    in_maps = _prep(inputs)
    res = run_bass_kernel_spmd(nc, in_maps, core_ids=list(range(B)), trace=trace)
    out = np.empty((S, B, C), np.float32)
    for b in range(B):
        out[:, b, :] = res.results[b]["y"].astype(np.float32)
    return out, res


def kernel(**inputs):
    out, _ = _run(inputs, trace=False)
    return out


# revision 5
# speedup vs baseline: 1.0470x; 1.0065x over previous
"""TRN2 Bass kernel for CompressedCausalAttention (batch-parallel, 8 cores).

Per-core dataflow (one batch element per NeuronCore):
  x^T, pe^T arrive pre-transposed from the host as [128, NK*S] bf16 chunk
  layouts; xpeT = x^T + pe^T via per-chunk DVE adds (no PE transposes).
  qkT [2C', S] = W_qk^T @ xpeT in bf16 (q bias folded into the PSUM->SBUF
  copy; k bias dropped entirely -- per-row-constant score shifts cancel in
  the softmax over t).
  V [S, C] bf16 with a ones column per head so the PV matmul accumulates
  softmax denominators in PSUM row 64 for free.
  Each pass p handles heads (2p, 2p+1) in two half-passes -- score columns
  [0,512) over t-blocks 0..3, then [512,1024) over t-blocks 0..7 -- so only
  two PV PSUM banks are live at once and the next pass's QK projection can
  run concurrently from its own PSUM pool (PSUM: 2 proj + 2x2 score + 2 PV
  banks). scores/exp/mask/PV all bf16: one exp per chunk covers both heads
  (strided 3D AP), gpsimd zeroes masked diagonal probs.
  Denominator reciprocals are taken directly off the PSUM row per
  (head, half); the rdenom broadcast over head rows happens via two rank-1
  0/1-row matmuls, deferred into the next pass's program order so the PSUM
  WAR never stalls. PSUM->SBUF copies are split across DVE (q, evict) and
  the scalar engine (k, V) to balance engine load; DMA issue is spread over
  the SP/Activation/Pool queues. Out projection in bf16; b_out (+ V-bias
  folded through W_out on the host) is added by the output copy against a
  host-broadcast [128, C] constant.
"""
import os
import numpy as np

import concourse.bass as bass
import concourse.bacc as bacc
import concourse.mybir as mybir
import concourse.tile as tile
from concourse.bass_utils import run_bass_kernel_spmd

S, B, C, H = 1024, 8, 768, 12
CC = C // H            # 64
NS = S // 128          # 8 s/t blocks
NK = C // 128          # 6 contraction chunks of 128
NKD = C // 256         # 3 DoubleRow chunks of 256
NM = 2 * C // 128      # 12 q+k M-tiles
F32 = mybir.dt.float32
BF16 = mybir.dt.bfloat16
FP8 = mybir.dt.float8e4
F32R = mybir.dt.float32r
REPEAT = int(os.environ.get("BASSK_REPEAT", "1"))
AF = mybir.ActivationFunctionType
ALU = mybir.AluOpType
DR = mybir.MatmulPerfMode.DoubleRow
WS = 64.0              # fp8 weight pre-scale
_CACHE = {}

# far-column chunks for the DR projections (N <= 256: rhs free = 2N <= 512)
FAR_CHUNKS = [(128, 384), (384, 640), (640, 896), (896, 1024)]


def _build(repeat=None):
    if repeat is None:
        repeat = REPEAT
    nc = bacc.Bacc("TRN2", target_bir_lowering=False, debug=False)

    Xb = nc.dram_tensor("xb", [128, NK * S], BF16, kind="ExternalInput")
    PEb = nc.dram_tensor("peb", [128, NK * S], BF16, kind="ExternalInput")
    Wqkb = nc.dram_tensor("wqkb", [128, NK * 2 * C], BF16, kind="ExternalInput")
    Wvb = nc.dram_tensor("wvb", [128, NK * C], BF16, kind="ExternalInput")
    Wo = nc.dram_tensor("wo", [128, NK * C], BF16, kind="ExternalInput")
    Bq = nc.dram_tensor("bq", [128, NK], F32, kind="ExternalInput")
    Beffb = nc.dram_tensor("beffb", [128, C], F32, kind="ExternalInput")
    Mask01 = nc.dram_tensor("mask01", [128, 128], BF16, kind="ExternalInput")
    E2W = nc.dram_tensor("e2w", [128, 128], BF16, kind="ExternalInput")
    OnesV = nc.dram_tensor("onesv", [128, H], BF16, kind="ExternalInput")
    Y = nc.dram_tensor("y", [S, C], BF16, kind="ExternalOutput")

    from contextlib import ExitStack
    with ExitStack() as _es:
        tc = _es.enter_context(tile.TileContext(nc))
        _p = lambda **kw: _es.enter_context(tc.tile_pool(**kw))
        cst = _p(name="cst", bufs=1)
        xin_p = _p(name="xin", bufs=1)
        qkT_p = _p(name="qkT", bufs=5)
        vx_p = _p(name="vx", bufs=NS)
        pT_p = _p(name="pT", bufs=4)
        pvT_p = _p(name="pvT", bufs=NK)
        ysb_p = _p(name="ysb", bufs=2)
        dstage_p = _p(name="dstage", bufs=2)
        ps1 = _p(name="ps1", bufs=2, space="PSUM")    # 2 x 1-bank slots
        scps = _p(name="scps", bufs=2, space="PSUM")  # 2 x 2-bank slots
        pvps = _p(name="pvps", bufs=2, space="PSUM")  # 2 x 1-bank slots

        def go():
            mask_sb = cst.tile([128, 128], BF16, tag="mask", name="mask_sb")
            bq_sb = cst.tile([128, NK], F32, tag="bq", name="bq_sb")
            beff_sb = cst.tile([128, C], F32, tag="beff", name="beff_sb")
            e2w_sb = cst.tile([128, 128], BF16, tag="e2w", name="e2w_sb")

            # ---- inputs: full bf16 transposed x/pe; add per k-chunk on DVE
            xb = xin_p.tile([128, NK * S], BF16, tag="xb", name="xb")
            peb = xin_p.tile([128, NK * S], BF16, tag="peb", name="peb")
            for kc in range(NK):
                sl = slice(kc * S, (kc + 1) * S)
                nc.scalar.dma_start(xb[:, sl], Xb.ap()[:, sl])
                nc.sync.dma_start(peb[:, sl], PEb.ap()[:, sl])
                nc.vector.tensor_tensor(xb[:, sl], xb[:, sl], peb[:, sl],
                                        ALU.add)
            xpeb = [xb[:, kc * S:(kc + 1) * S] for kc in range(NK)]

            # ---- weights
            wqkb_sb = cst.tile([128, NK * 2 * C], BF16, tag="wqkb", name="wqkb_sb")
            nc.sync.dma_start(wqkb_sb[:], Wqkb.ap())
            wvb_sb = cst.tile([128, NK * C], BF16, tag="wvb", name="wvb_sb")
            nc.gpsimd.dma_start(wvb_sb[:], Wvb.ap())
            wo_sb = cst.tile([128, NK * C], BF16, tag="wo", name="wo_sb")
            nc.sync.dma_start(wo_sb[:], Wo.ap())
            nc.gpsimd.dma_start(mask_sb[:], Mask01.ap())
            nc.gpsimd.dma_start(bq_sb[:], Bq.ap())
            nc.sync.dma_start(beff_sb[:], Beffb.ap())
            nc.sync.dma_start(e2w_sb[:], E2W.ap())
            wqkbv = [wqkb_sb[:, kc * 2 * C:(kc + 1) * 2 * C] for kc in range(NK)]
            wvbv = [wvb_sb[:, kc * C:(kc + 1) * C] for kc in range(NK)]
            wov = [wo_sb[:, kc * C:(kc + 1) * C] for kc in range(NK)]

            # ---- Phase A2: V (near: block 0 bf16; far: blocks 1..7 fp8 DR)
            vx = []
            for si in range(NS):
                v = vx_p.tile([128, H * (CC + 1)], BF16, tag="vx", name=f"vx{si}")
                v3 = v[:].rearrange("p (h c) -> p h c", h=H)

                def vcopy(ps_, c0, c1, scale):
                    h0, h1 = c0 // CC, c1 // CC
                    nc.scalar.activation(
                        v3[:, h0:h1, 0:CC],
                        ps_[:].rearrange("p (h c) -> p h c", h=h1 - h0),
                        AF.Copy, scale=scale)
                for (c0, c1) in ((0, 512), (512, C)):
                    vp = ps1.tile([128, c1 - c0], F32, tag="ps1",
                                  name=f"vp{si}_{c0}")
                    for k in range(NK):
                        nc.tensor.matmul(
                            vp[:], xpeb[k][:, si * 128:(si + 1) * 128],
                            wvbv[k][:, c0:c1],
                            start=(k == 0), stop=(k == NK - 1))
                    vcopy(vp, c0, c1, 1.0)
                nc.gpsimd.dma_start(v3[:, :, CC:CC + 1], OnesV.ap()[:, :, None])
                vx.append(v)

            # ---- Phases B+C: per pass p: qkT tiles (p, 6+p), heads 2p, 2p+1
            pvT = [pvT_p.tile([128, S], BF16, tag="pvT", name=f"pvT{j}")
                   for j in range(NK)]
            def run_norm(job):
                # bc + multiply for a finished pass; placed after the next
                # pass's projection allocs so the ps1 WAR hits a fast slot
                p_, rden2_ = job
                for n2 in (0, 1):
                    bc = ps1.tile([128, 512], F32, tag="ps1",
                                  name=f"bc{p_}_{n2}")
                    nc.tensor.matmul(
                        bc[:], e2w_sb[:],
                        rden2_[:, n2 * 512:(n2 + 1) * 512],
                        start=True, stop=True)
                    nc.vector.tensor_tensor(
                        pvT[p_][:, n2 * 512:(n2 + 1) * 512],
                        pvT[p_][:, n2 * 512:(n2 + 1) * 512], bc[:], ALU.mult)

            norm_job = None
            for p in range(6):
                qk = {}
                for mm in (p, 6 + p):
                    t = qkT_p.tile([128, S], BF16, tag="qkT", name=f"qkT{mm}")
                    is_q = mm < NK
                    for (a0, a1) in ((0, 512), (512, S)):
                        psf = ps1.tile([128, 512], F32, tag="ps1",
                                       name=f"qf{mm}_{a0}")
                        for k in range(NK):
                            nc.tensor.matmul(
                                psf[:], wqkbv[k][:, mm * 128:(mm + 1) * 128],
                                xpeb[k][:, a0:a1],
                                start=(k == 0), stop=(k == NK - 1))
                        if is_q:
                            nc.vector.tensor_scalar(
                                t[:, a0:a1], psf[:], bq_sb[:, mm:mm + 1],
                                None, ALU.add)
                        else:
                            nc.scalar.activation(t[:, a0:a1], psf[:], AF.Copy)
                    qk[mm] = t
                if norm_job is not None:
                    run_norm(norm_job)
                    norm_job = None
                qt, kt = qk[p], qk[6 + p]
                heads = (2 * p, 2 * p + 1)
                rden2 = dstage_p.tile([128, S], BF16, tag="rdst",
                                      name=f"rden2_{p}")
                nc.gpsimd.memset(rden2[:], 0.0)

                def evict(pvt, h, half):
                    r0 = (h % 2) * CC
                    rr = (h % 2) * 64
                    c0 = half * 512
                    with nc.allow_low_precision(reason="bf16 softmax scale"):
                        nc.vector.reciprocal(rden2[rr:rr + 1, c0:c0 + 512],
                                             pvt[CC:CC + 1, :])
                    nc.vector.tensor_copy(pvT[p][r0:r0 + CC,
                                                 half * 512:half * 512 + 512],
                                          pvt[0:CC, :])

                # two half-passes: cols [0,512) over Ti 0..3, then cols
                # [512,1024) over Ti 0..7 — only 2 PV banks live at a time
                for half, tis in ((0, range(4)), (1, range(NS))):
                    c0 = half * 512
                    pv = {h: pvps.tile([CC + 1, 512], F32, tag="pvps",
                                       name=f"pv{h}_{half}")
                          for h in heads}
                    for Ti in tis:
                        s0 = Ti * 128
                        a0 = max(s0, c0)
                        a1 = c0 + 512
                        w = a1 - a0
                        pt = pT_p.tile([128, 2 * w], BF16, tag="pT",
                                       name=f"pt{p}_{half}_{Ti}")
                        off = {heads[0]: 0, heads[1]: w}
                        sc = scps.tile([128, 1024], F32, tag="scps",
                                       name=f"sc{p}_{half}_{Ti}")
                        for hi, h in enumerate(heads):
                            r0 = (h % 2) * CC
                            nc.tensor.matmul(
                                sc[:, hi * 512:hi * 512 + w],
                                kt[r0:r0 + CC, s0:s0 + 128],
                                qt[r0:r0 + CC, a0:a1],
                                start=True, stop=True)
                        ap_out = pt[:].rearrange("q (i c) -> q i c", i=2)
                        ap_in = sc[:].rearrange(
                            "q (i c) -> q i c", i=2)[:, :, 0:w]
                        nc.scalar.activation(
                            ap_out, ap_in, AF.Exp, scale=float(1.0 / np.sqrt(CC)))
                        if a0 == s0:  # diagonal blocks: zero masked probs
                            for h in heads:
                                o = off[h]
                                nc.gpsimd.tensor_tensor(
                                    pt[:, o:o + 128], pt[:, o:o + 128],
                                    mask_sb[:], ALU.mult)
                        for h in heads:
                            v3 = vx[Ti][:].rearrange("p (h c) -> p h c", h=H)
                            nc.tensor.matmul(
                                pv[h][:, a0 - c0:a1 - c0], v3[:, h, :],
                                pt[:, off[h]:off[h] + w],
                                start=(Ti == tis[0]), stop=(Ti == tis[-1]))
                    for h in heads:
                        evict(pv[h], h, half)

                # bc deferred to next pass
                norm_job = (p, rden2)

            # ---- Phase D: output projection (last pass's norm slots in
            # after the first psum group's k<5 matmuls to hide its stall)
            for si in range(NS):
                ty = ysb_p.tile([128, C], BF16, tag="ysb", name=f"ty{si}")
                for (c0, c1) in ((0, 512), (512, C)):
                    yp = ps1.tile([128, c1 - c0], F32, tag="ps1",
                                  name=f"yp{si}_{c0}")
                    for k in range(NK):
                        if norm_job is not None and k == NK - 1:
                            run_norm(norm_job)
                            norm_job = None
                        nc.tensor.matmul(
                            yp[:], pvT[k][:, si * 128:(si + 1) * 128],
                            wov[k][:, c0:c1],
                            start=(k == 0), stop=(k == NK - 1))
                    nc.vector.tensor_tensor(ty[:, c0:c1], yp[:],
                                            beff_sb[:, c0:c1], ALU.add)
                    nc.sync.dma_start(
                        Y.ap()[si * 128:(si + 1) * 128, c0:c1], ty[:, c0:c1])

        for _rep in range(repeat):
            go()

    nc.compile()
    return nc


def _prep(inputs):
    np8 = mybir.dt.np(FP8)
    npb = mybir.dt.np(BF16)
    x = np.asarray(inputs["x"], np.float32)
    pe = np.asarray(inputs["pe"], np.float32)
    W_qkv = np.asarray(inputs["W_qkv"], np.float32)
    b_qkv = np.asarray(inputs["b_qkv"], np.float32)
    W_out = np.asarray(inputs["W_out"], np.float32)
    b_out = np.asarray(inputs["b_out"], np.float32)

    wqk = np.ascontiguousarray(W_qkv[:, :2 * C])          # [C, 2C]
    wv = np.ascontiguousarray(W_qkv[:, 2 * C:])           # [C, C]
    wqkb = wqk.reshape(NK, 128, 2 * C).transpose(1, 0, 2) \
        .reshape(128, NK * 2 * C).astype(npb)
    wvb = wv.reshape(NK, 128, C).transpose(1, 0, 2) \
        .reshape(128, NK * C).astype(npb)
    wo = W_out.reshape(NK, 128, C).transpose(1, 0, 2) \
        .reshape(128, NK * C).astype(npb)
    bq = np.ascontiguousarray(b_qkv[:C].reshape(NK, 128).T).astype(np.float32)
    beff = (b_qkv[2 * C:] @ W_out + b_out).astype(np.float32)
    beffb = np.ascontiguousarray(np.broadcast_to(beff[None, :], (128, C)))
    t = np.arange(128)
    mask01 = (t[:, None] <= t[None, :]).astype(npb)
    e2w = np.zeros((128, 128), np.float32)
    e2w[0, :CC] = 1.0
    e2w[CC, CC:] = 1.0
    common = dict(wqkb=wqkb, wvb=wvb, wo=wo, bq=bq,
                  beffb=beffb, mask01=mask01, e2w=e2w.astype(npb),
                  onesv=np.ones((128, H), npb))

    in_maps = []
    for b in range(B):
        m = dict(common)
        xT = x[:, b, :].T                                  # [C, S]
        peT = pe[:, b, :].T
        m["xb"] = np.ascontiguousarray(
            xT.reshape(NK, 128, S).transpose(1, 0, 2)
            .reshape(128, NK * S)).astype(npb)
        m["peb"] = np.ascontiguousarray(
            peT.reshape(NK, 128, S).transpose(1, 0, 2)
            .reshape(128, NK * S)).astype(npb)
        in_maps.append(m)
    return in_maps


def _run(inputs, trace=False):
    if "nc" not in _CACHE:
        _CACHE["nc"] = _build()
    nc = _CACHE["nc"]
    in_maps = _prep(inputs)
    res = run_bass_kernel_spmd(nc, in_maps, core_ids=list(range(B)), trace=trace)
    out = np.empty((S, B, C), np.float32)
    for b in range(B):
        out[:, b, :] = res.results[b]["y"].astype(np.float32)
    return out, res


def kernel(**inputs):
    out, _ = _run(inputs, trace=False)
    return out
